# revision 1
# baseline (speedup 1.0000x reference)
"""MoE (BruteForceMoELinear) Trainium2 kernel.

Strategy: expert-parallel across 8 NeuronCores. The host (inside
`kernel()`) dispatches token rows by `gate_idx` (stable sort), pads each
expert's token batch to a common capacity C, and hands core e:

  xt  : (128, 4, C)    = x_e^T   laid out [d_inner, d_outer, token]
  w1t : (128, 4, 2048) = W1_e^T  laid out [d_inner, d_outer, f]
  w2t : (128, 16, 512) = W2_e^T  laid out [f_inner, f_outer, d_out]
  sc  : (128, C)       = per-token gate score, replicated over partitions

Each core computes  y_e^T = (W2_e @ relu(W1_e @ x_e^T)) * score  with
float32r matmuls (full-rate fp32 PE path), ReLU fused into the PSUM
eviction on the scalar engine and the gate-score multiply fused into the
second GEMM's PSUM eviction on the vector engine.  The host scatters the
per-expert outputs back to token order and sums the top-k (=2) slots.
"""

import numpy as np

NUM_EXPERT = 8
N_CORES = 8
P = 128

_CACHE = {}


def _build(TN, NCH, KO, FO, repeat=1):
    """Compile the per-core program for capacity C = TN*NCH tokens.

    KO = d_model/128, FO = d_ff/128.  `repeat` re-emits the compute body
    (used only for timing calibration in the dev harness).
    """
    key = (TN, NCH, KO, FO, repeat)
    if key in _CACHE:
        return _CACHE[key]

    import concourse.mybir as mybir
    import concourse.tile as tile
    from concourse import bacc

    f32 = mybir.dt.float32
    f32r = mybir.dt.float32r
    C = TN * NCH
    D_MODEL = KO * P
    D_FF = FO * P

    nc = bacc.Bacc("TRN2", target_bir_lowering=False, debug=False,
                   num_devices=N_CORES)

    xt = nc.dram_tensor("xt", (P, KO, C), f32r, kind="ExternalInput")
    w1t = nc.dram_tensor("w1t", (P, KO, D_FF), f32r, kind="ExternalInput")
    w2t = nc.dram_tensor("w2t", (P, FO, D_MODEL), f32r, kind="ExternalInput")
    sc = nc.dram_tensor("sc", (P, C), f32, kind="ExternalInput")
    yt = nc.dram_tensor("yt", (P, KO, C), f32, kind="ExternalOutput")

    # Holding every chunk's h in SBUF only fits for NCH <= 2; for heavily
    # skewed expert distributions (NCH >= 3) process chunk-major with a
    # rotating 2-buffer h pool instead.
    # NOTE: pools reserve bufs slots PER TAG; the NCH<=2 path uses one
    # persistent tile per chunk tag, so 1 slot per tag suffices (bufs=NCH
    # would double-reserve and overflow SBUF around TN>=400, NCH=2).
    NHB = 1 if NCH <= 2 else 2
    NXB = 1 if NCH <= 2 else 3
    with tile.TileContext(nc) as tc:
        with tc.tile_pool(name="wpool", bufs=1) as wpool, \
             tc.tile_pool(name="xpool", bufs=NXB) as xpool, \
             tc.tile_pool(name="hpool", bufs=NHB) as hpool, \
             tc.tile_pool(name="ypool", bufs=4) as ypool, \
             tc.tile_pool(name="cpool", bufs=1) as cpool, \
             tc.tile_pool(name="ps1", bufs=6, space="PSUM") as ps1, \
             tc.tile_pool(name="ps2", bufs=2, space="PSUM") as ps2:

            bias0 = cpool.tile([P, 1], f32)
            nc.any.memset(bias0[:], 0.0)

            # PE warm-up: dummy matmuls on memset data keep the PE busy
            # through the DMA-priming window so the HAM clock gate is at
            # full rate when the first real matmul issues.
            warm = cpool.tile([P, 64], f32)
            nc.any.memset(warm[:], 0.5)
            wps = ps1.tile([P, 64], f32, name="warm", tag="p1")
            for _i in range(20):
                nc.tensor.matmul(wps[:64, :], warm[:], warm[:],
                                 start=True, stop=True)

            # DMAs execute in emission order on the DMA stream, which is
            # the pacing resource at kernel start.  Emit strictly in
            # consumption order: x(ch0) -> W1 -> x(ch1..) -> W2/sc.
            w1sb = wpool.tile([P, KO, D_FF], f32r)
            w2sb = wpool.tile([P, FO, D_MODEL], f32r)
            scsb = cpool.tile([P, C], f32)
            if NCH <= 2:
                xsbs = [xpool.tile([P, KO, TN], f32r, tag=f"x{ch}",
                                   name=f"xsb{ch}") for ch in range(NCH)]
            else:
                xsbs = None  # allocated per chunk in the fallback loop

            # DMA emission order == consumption order: x/W1 for the first
            # f-block pairwise (fine-grained so the first fo-group starts
            # after ~3 small DMAs), later chunks' x, the rest of W1, then
            # W2 d-blocks and the gate scores.
            FB = 512
            NFB = D_FF // FB
            FPB = FB // P  # fo-groups per W1 f-block
            if NCH <= 2:
                nc.sync.dma_start(w1sb[:, 0:2, 0:FB],
                                  w1t.ap()[:, 0:2, 0:FB])
                nc.sync.dma_start(xsbs[0][:], xt.ap()[:, :, 0:TN])
                nc.sync.dma_start(w1sb[:, 2:KO, 0:FB],
                                  w1t.ap()[:, 2:KO, 0:FB])
                for ch in range(1, NCH):
                    nc.sync.dma_start(xsbs[ch][:],
                                      xt.ap()[:, :, ch * TN:(ch + 1) * TN])
            else:
                nc.sync.dma_start(w1sb[:, :, 0:FB], w1t.ap()[:, :, 0:FB])
            HB = FB // 4
            for hb in range(4, 4 * NFB):
                nc.sync.dma_start(
                    w1sb[:, :, hb * HB:(hb + 1) * HB],
                    w1t.ap()[:, :, hb * HB:(hb + 1) * HB])
            nc.sync.dma_start(w2sb[:, :, 0:P], w2t.ap()[:, :, 0:P])
            nc.sync.dma_start(scsb[:], sc.ap())
            for db in range(1, KO):
                nc.sync.dma_start(w2sb[:, :, db * P:(db + 1) * P],
                                  w2t.ap()[:, :, db * P:(db + 1) * P])

            def gemm1(hsb, xsb, fo):
                p1 = ps1.tile([P, TN], f32, name="p1", tag="p1")
                for ko in range(KO):
                    nc.tensor.matmul(
                        p1[:],
                        w1sb[:, ko, fo * P:(fo + 1) * P],
                        xsb[:, ko, :],
                        start=(ko == 0), stop=(ko == KO - 1))
                nc.scalar.activation(
                    hsb[:, fo, :], p1[:],
                    mybir.ActivationFunctionType.Relu, bias=bias0[:])

            def gemm2(hsb, do, tsl):
                p2 = ps2.tile([P, TN], f32, name="p2", tag="p2")
                for fo in range(FO):
                    nc.tensor.matmul(
                        p2[:],
                        w2sb[:, fo, do * P:(do + 1) * P],
                        hsb[:, fo, :],
                        start=(fo == 0), stop=(fo == FO - 1))
                ysb = ypool.tile([P, TN], f32, tag="y", name="ysb")
                nc.vector.tensor_mul(ysb[:], p2[:], scsb[:, tsl])
                nc.sync.dma_start(yt.ap()[:, do, tsl], ysb[:])

            for _ in range(repeat):
                if NCH <= 2:
                    hsbs = [hpool.tile([P, FO, TN], f32r, tag=f"h{ch}",
                                       name=f"hsb{ch}") for ch in range(NCH)]
                    # phase 1: h = relu(W1 @ x^T); f-block-major so every
                    # W1 block feeds all chunks' matmuls before the next
                    # block is needed (keeps PE ahead of the DMA stream).
                    for fb in range(NFB):
                        for ch in range(NCH):
                            for fo in range(fb * FPB, (fb + 1) * FPB):
                                gemm1(hsbs[ch], xsbs[ch], fo)
                    # phase 2: y^T = (W2 @ h) * score; d-block-major,
                    # streamed out per (db, chunk).
                    for do in range(KO):
                        for ch in range(NCH):
                            gemm2(hsbs[ch], do,
                                  slice(ch * TN, (ch + 1) * TN))
                else:
                    # chunk-major fallback (bounded SBUF for large NCH)
                    for ch in range(NCH):
                        xsb = xpool.tile([P, KO, TN], f32r, tag="x",
                                         name="xsb")
                        nc.sync.dma_start(
                            xsb[:], xt.ap()[:, :, ch * TN:(ch + 1) * TN])
                        hsb = hpool.tile([P, FO, TN], f32r, tag="h",
                                         name="hsb")
                        for fo in range(FO):
                            gemm1(hsb, xsb, fo)
                        for do in range(KO):
                            gemm2(hsb, do, slice(ch * TN, (ch + 1) * TN))

    nc.compile()
    _CACHE[key] = nc
    return nc


def _capacity(max_count):
    """Chunking: NCH chunks of TN tokens; TN in [256, 512] keeps the
    float32r matmul at full rate and within one PSUM bank."""
    maxc = max(int(max_count), 1)
    nch = -(-maxc // 512)
    tn = -(-maxc // (nch * 8)) * 8
    tn = max(tn, 256)
    return tn, nch


_last = {}


def kernel(inp, gate_idx, gate_score, w_htoh4, w_h4toh):
    inp = np.ascontiguousarray(np.asarray(inp, dtype=np.float32))
    gate_idx = np.asarray(gate_idx)
    gate_score = np.asarray(gate_score, dtype=np.float32)
    w_htoh4 = np.asarray(w_htoh4, dtype=np.float32)
    w_h4toh = np.asarray(w_h4toh, dtype=np.float32)

    B, d_model = inp.shape
    n_expert, d_ff, _ = w_htoh4.shape
    assert n_expert == NUM_EXPERT
    KO = d_model // P
    FO = d_ff // P

    gi = gate_idx.astype(np.int64)
    order = np.argsort(gi, kind="stable")
    counts = np.bincount(gi, minlength=NUM_EXPERT)
    idx_split = np.split(order, np.cumsum(counts)[:-1])

    TN, NCH = _capacity(counts.max())
    C = TN * NCH

    # flat per-row gate scores: row 2n+k of inp gets gate_score[n, 0, k]
    scores_flat = gate_score.reshape(-1)

    nc = _build(TN, NCH, KO, FO)

    in_maps = []
    for e in range(NUM_EXPERT):
        idx = idx_split[e]
        cnt = len(idx)
        xT = np.zeros((d_model, C), dtype=np.float32)
        if cnt:
            xT[:, :cnt] = inp[idx].T
        xt_h = np.ascontiguousarray(
            xT.reshape(KO, P, C).transpose(1, 0, 2))
        w1_h = np.ascontiguousarray(
            w_htoh4[e].T.reshape(KO, P, d_ff).transpose(1, 0, 2))
        w2_h = np.ascontiguousarray(
            w_h4toh[e].T.reshape(FO, P, d_model).transpose(1, 0, 2))
        sc_vec = np.zeros((C,), dtype=np.float32)
        if cnt:
            sc_vec[:cnt] = scores_flat[idx]
        sc_h = np.ascontiguousarray(np.broadcast_to(sc_vec, (P, C)))
        in_maps.append({"xt": xt_h, "w1t": w1_h, "w2t": w2_h, "sc": sc_h})

    from concourse import bass_utils
    res = bass_utils.run_bass_kernel_spmd(nc, in_maps,
                                          core_ids=list(range(N_CORES)))

    _last.update(nc=nc, in_maps=in_maps, res=res, TN=TN, NCH=NCH,
                 KO=KO, FO=FO)

    y_full = np.empty((B, d_model), dtype=np.float32)
    for e in range(NUM_EXPERT):
        idx = idx_split[e]
        if len(idx) == 0:
            continue
        yt_h = res.results[e]["yt"]  # (P, KO, C)
        yT = yt_h.transpose(1, 0, 2).reshape(d_model, C)
        y_full[idx] = yT[:, :len(idx)].T

    out = y_full[0::2] + y_full[1::2]
    return np.ascontiguousarray(out, dtype=np.float32)



# revision 61
# speedup vs baseline: 1.2203x; 1.2203x over previous
"""MoE (BruteForceMoELinear) Trainium2 kernel — fp8 DoubleRow expert-parallel.

Strategy: expert-parallel across 8 NeuronCores.  The host dispatches
token rows by `gate_idx` (stable sort), folds the per-row gate score
into the token vector (ReLU is positively homogeneous), pads each
expert's batch to capacity C = sum(chunks), and quantizes operands to
fp8-e4m3 with residual compensation:

  x  = xh + xl            (xh, xl fp8; xl = residual, fp8-exact range)
  W1 = W1h + W1l_s/S      (W1l_s = S*(W1-W1h) in fp8 — scaled so the
                           residual stays out of the subnormal range)
  W2 = W2h + W2l_s/S

Each GEMM runs 3 compensated fp8 DoubleRow matmul terms (K=256 per
instruction, 0.5 PE cycles/row — 2x the bf16 rate):

  h  = relu(xh@W1h + xl@W1h + (xh/S)@W1l_s)          [PSUM f32 accum]
  y  = hh@W2h + hl@W2h + (hh/S)@W2l_s                [PSUM f32 accum]

with hh = fp8(relu(p1)) on the scalar engine, hl = fp8(relu(p1)-hh) on
the vector engine (scalar_tensor_tensor), and hh/S on the gpsimd
engine (exact power-of-2 shift).  Measured end-to-end rel-err vs the
fp32 reference: ~3e-3 (tolerance 2e-2).

The host scatters per-expert outputs back to token order and sums the
top-k (=2) slots.
"""

import numpy as np
import ml_dtypes

BF16 = ml_dtypes.bfloat16
F8 = ml_dtypes.float8_e4m3   # matches mybir.dt.float8e4

NUM_EXPERT = 8
N_CORES = 8
P = 128
D_MODEL = 512
D_FF = 2048
KO = D_MODEL // P   # 4
FO = D_FF // P      # 16
S = 16.0            # residual scale (power of 2 — exact in fp8)

_CACHE = {}


def _build(chunks, repeat=1, n_warm=20, intro=None, split_last=0):
    """Compile the per-core program for capacity C = sum(chunks)."""
    if intro is None:
        # pair-step at which each chunk joins GEMM1 (chunk 0 leads, the
        # tail chunk last) — gives each x-chunk DMA time to land
        intro = tuple(0 if ch == 0 else (1 if ch == 1 else FO // 2 - 1)
                      for ch in range(len(chunks)))
    key = (tuple(chunks), repeat, n_warm, tuple(intro), split_last)
    if key in _CACHE:
        return _CACHE[key]

    import concourse.mybir as mybir
    import concourse.tile as tile
    from concourse import bacc

    f32 = mybir.dt.float32
    bf16 = mybir.dt.bfloat16
    fp8 = mybir.dt.float8e4
    DR = mybir.MatmulPerfMode.DoubleRow
    AOP = mybir.AluOpType
    C = sum(chunks)
    NCH = len(chunks)
    starts = [sum(chunks[:i]) for i in range(NCH)]

    nc = bacc.Bacc("TRN2", target_bir_lowering=False, debug=False,
                   num_devices=N_CORES)

    # merged operand tensors: one DMA feeds all three compensation
    # terms.  fp8 elements are 1 byte, so layouts keep the innermost
    # contiguous DRAM runs >= 512B (else the DMA pays a 2x latency
    # penalty): x is per-chunk (whole-tensor DMA), w1 is f-blocked.
    # xq* dim1: [xh ko0..3 | xl ko0..3 | xs ko0..3]
    # w1q: (P, f-block, [w1h ko0..3 | w1l ko0..3], 256)
    # w2q dim2: [w2h | w2l]
    NB = D_FF // 256
    xqs = [nc.dram_tensor(f"xq{ch}", (P, 3 * KO, chunks[ch]), fp8,
                          kind="ExternalInput") for ch in range(NCH)]
    w1q = nc.dram_tensor("w1q", (P, NB, 2 * KO, 256), fp8,
                         kind="ExternalInput")
    w2q = nc.dram_tensor("w2q", (P, KO, 2, FO, P), fp8,
                         kind="ExternalInput")
    yt = nc.dram_tensor("yt", (P, KO, C), bf16, kind="ExternalOutput")

    with tile.TileContext(nc) as tc:
        with tc.tile_pool(name="wpool", bufs=1) as wpool, \
             tc.tile_pool(name="xpool", bufs=1) as xpool, \
             tc.tile_pool(name="hpool", bufs=1) as hpool, \
             tc.tile_pool(name="ypool", bufs=4) as ypool, \
             tc.tile_pool(name="cpool", bufs=1) as cpool, \
             tc.tile_pool(name="ps1", bufs=4, space="PSUM") as ps1, \
             tc.tile_pool(name="ps2", bufs=3, space="PSUM") as ps2:

            bias0 = cpool.tile([P, 1], f32)
            nc.any.memset(bias0[:], 0.0)

            # PE warm-up through the DMA-priming window + Relu act-table
            # warm (reads SBUF — no coupling with the PE warm-up loop).
            # Wide (256-col) warm matmuls keep the p-state ramp alive to
            # the first real matmul with few instructions.
            warm = cpool.tile([P, 256], bf16)
            nc.vector.memset(warm[:], 0.5)
            warmh = cpool.tile([P, 64], bf16)
            nc.scalar.activation(warmh[:], warm[:, 0:64],
                                 mybir.ActivationFunctionType.Relu,
                                 bias=bias0[:])
            wps = ps2.tile([P, 2, 256], f32, name="warm", tag="p2")
            for _i in range(n_warm):
                nc.tensor.matmul(wps[:, 0, :], warm[:, 0:128], warm[:],
                                 start=True, stop=True)

            w1sb = wpool.tile([P, NB, 2 * KO, 256], fp8)
            w2sb = wpool.tile([P, KO, 2, FO, P], fp8)
            xsbs = [xpool.tile([P, 3 * KO, chunks[ch]], fp8, tag=f"x{ch}",
                               name=f"xsb{ch}") for ch in range(NCH)]

            # DMA emission order == consumption order.  256-f W1 pieces:
            # transfer time (~0.73us) just above the HWDGE descriptor-gen
            # serialization (~0.63us per DMA), and each piece exactly
            # feeds one fo-pair GEMM1 group.
            nc.sync.dma_start(xsbs[0][:], xqs[0].ap())
            for b in range(NB):
                nc.sync.dma_start(w1sb[:, b], w1q.ap()[:, b])
                if b == 1:
                    for ch in range(1, NCH):
                        nc.sync.dma_start(xsbs[ch][:], xqs[ch].ap())
            for db in range(KO):
                nc.sync.dma_start(w2sb[:, db], w2q.ap()[:, db])

            hhs = [hpool.tile([P, FO, chunks[ch]], fp8, tag=f"hh{ch}",
                              name=f"hh{ch}") for ch in range(NCH)]
            hls = [hpool.tile([P, FO, chunks[ch]], fp8, tag=f"hl{ch}",
                              name=f"hl{ch}") for ch in range(NCH)]

            def gemm1_pair(ch, fo, n_fo):
                # fo-QUAD group: one PSUM tile [P, n_fo, tn] (2 banks at
                # tn=256; each 1KB fo-slice stays inside a bank) holds
                # n_fo accumulation groups; ONE wide eviction per engine
                # amortizes the per-instruction PSUM access latency —
                # necessary because DVE's hl pass must keep up with the
                # fp8-DR PE rate.
                tn = chunks[ch]
                p1 = ps1.tile([P, n_fo, tn], f32, name="p1", tag="p1")
                terms = [(0, 0), (0, KO), (KO, 2 * KO)]
                for j in range(n_fo):
                    b, half = (fo + j) // 2, (fo + j) % 2
                    n = 0
                    for wo, xo in terms:
                        for t in range(KO // 2):
                            nc.tensor.matmul(
                                p1[:, j, :],
                                w1sb[:, b, wo + 2 * t:wo + 2 * t + 2,
                                     half * P:(half + 1) * P],
                                xsbs[ch][:, xo + 2 * t:xo + 2 * t + 2, :],
                                start=(n == 0),
                                stop=(n == 3 * (KO // 2) - 1),
                                perf_mode=DR)
                            n += 1
                nc.scalar.activation(
                    hhs[ch][:, fo:fo + n_fo, :], p1[:],
                    mybir.ActivationFunctionType.Relu, bias=bias0[:])
                # hl = fp8(max(p1,0) - hh) on DVE (gpsimd's
                # scalar_tensor_tensor fails the walrus verifier)
                nc.vector.scalar_tensor_tensor(
                    hls[ch][:, fo:fo + n_fo, :], p1[:], 0.0,
                    hhs[ch][:, fo:fo + n_fo, :], AOP.max, AOP.subtract)

            store_q = [nc.sync, nc.scalar]
            store_i = [0]

            def gemm2(ch, db, t0, tn, csl):
                # all three terms accumulate S-scaled in ONE PSUM tile
                # (host stores W2h*S and W2l*S — exact fp8 exponent
                # shifts); the single 1/S rescale rides the activation
                # engine's built-in output scale at eviction.
                p2 = ps2.tile([P, tn], f32, name="p2", tag="p2")
                for n, (hsb, wi) in enumerate(
                        ((hhs[ch], 0), (hls[ch], 0), (hhs[ch], 1))):
                    for g in range(FO // 2):
                        nc.tensor.matmul(
                            p2[:],
                            w2sb[:, db, wi, 2 * g:2 * g + 2, :],
                            hsb[:, 2 * g:2 * g + 2, t0:t0 + tn],
                            start=(g == 0 and n == 0),
                            stop=(g == FO // 2 - 1 and n == 2),
                            perf_mode=DR)
                ysb = ypool.tile([P, tn], bf16, tag="y", name="ysb")
                nc.scalar.mul(ysb[:], p2[:], 1.0 / S)
                q = store_q[store_i[0] % len(store_q)]
                store_i[0] += 1
                q.dma_start(yt.ap()[:, db, csl], ysb[:])

            for _ in range(repeat):
                # phase 1: quad-step-major with staggered chunk intro
                # (chunk ch joins at quad-step intro[ch] and catches up
                # on its backlog).
                emitted = [0] * NCH
                for step in range(FO // 2):
                    for ch in range(NCH):
                        if intro[ch] <= step:
                            while emitted[ch] <= step:
                                gemm1_pair(ch, 2 * emitted[ch], 2)
                                emitted[ch] += 1
                for ch in range(NCH):
                    while emitted[ch] < FO // 2:
                        gemm1_pair(ch, 2 * emitted[ch], 2)
                        emitted[ch] += 1
                # phase 2: d-block-major, streamed out.  The final
                # d-block runs small chunks first and splits the last
                # big chunk (tn-64, 64) so the very last store's DMA
                # chain is short.
                for db in range(KO):
                    order = list(range(NCH))
                    if db == KO - 1 and split_last:
                        order.sort(key=lambda c: -chunks[c])
                        order = order[1:][::-1] + order[:1]
                    for ch in order:
                        tn = chunks[ch]
                        s0 = starts[ch]
                        if (db == KO - 1 and split_last and tn > 128
                                and ch == order[-1]):
                            t1 = tn - 64
                            gemm2(ch, db, 0, t1, slice(s0, s0 + t1))
                            gemm2(ch, db, t1, 64,
                                  slice(s0 + t1, s0 + tn))
                        else:
                            gemm2(ch, db, 0, tn, slice(s0, s0 + tn))

    nc.compile()
    _CACHE[key] = nc
    return nc


def _capacity(max_count):
    """C >= max_count, multiple of 8; 256-token chunks (a 2-fo PSUM
    tile fills one 2KB bank exactly) plus a small remainder chunk."""
    maxc = max(int(max_count), 16)
    c = -(-maxc // 8) * 8
    chunks = []
    while c > 256:
        chunks.append(256)
        c -= 256
    chunks.append(c)
    return tuple(chunks)


def _q8(x):
    return np.asarray(x, dtype=np.float32).astype(F8)


def _pack_expert(xT, w1, w2, chunks):
    """Quantize + lay out one expert's operands.

    xT: (d_model, C) f32 (scores folded).  Returns the in_map with the
    per-chunk xq* plus w1q / w2q tensors (see _build for dim layouts).
    """
    C = xT.shape[1]
    assert sum(chunks) == C
    xh_f = _q8(xT)
    xl_f = _q8(xT - xh_f.astype(np.float32))
    xs_f = _q8(xh_f.astype(np.float32) / S)

    def pack_x(m):  # (d_model, C) -> (P, KO, C)
        return m.reshape(KO, P, C).transpose(1, 0, 2)

    xq = np.concatenate(
        [pack_x(xh_f), pack_x(xl_f), pack_x(xs_f)], axis=1)
    in_map = {}
    s0 = 0
    for ch, tn in enumerate(chunks):
        in_map[f"xq{ch}"] = np.ascontiguousarray(xq[:, :, s0:s0 + tn])
        s0 += tn

    # w1_h[p, ko, f] = W1[f, ko*128+p]; blocked (P, NB, 2*KO, 256)
    w1T = w1.T.reshape(KO, P, D_FF).transpose(1, 0, 2)
    w1h_f = _q8(w1T)
    w1l_f = _q8(S * (w1T - w1h_f.astype(np.float32)))
    w1cat = np.concatenate([w1h_f, w1l_f], axis=1)  # (P, 8, D_FF)
    NB = D_FF // 256
    in_map["w1q"] = np.ascontiguousarray(
        w1cat.reshape(P, 2 * KO, NB, 256).transpose(0, 2, 1, 3))

    # w2_h[p, db, fo, q] = W2[db*128+q, fo*128+p]; both W2 operands are
    # stored pre-scaled by S (exact fp8 exponent shift) so GEMM2's
    # three terms share one PSUM accumulation; eviction divides by S.
    w2T = w2.T.reshape(FO, P, KO, P).transpose(1, 2, 0, 3)
    w2h_f = _q8(w2T)
    w2hs_f = _q8(S * w2h_f.astype(np.float32))
    w2l_f = _q8(S * (w2T - w2h_f.astype(np.float32)))
    in_map["w2q"] = np.ascontiguousarray(
        np.stack([w2hs_f, w2l_f], axis=2))  # (P, KO, 2, FO, P)

    return in_map


_last = {}


def kernel(inp, gate_idx, gate_score, w_htoh4, w_h4toh):
    inp = np.ascontiguousarray(np.asarray(inp, dtype=np.float32))
    gate_idx = np.asarray(gate_idx)
    gate_score = np.asarray(gate_score, dtype=np.float32)
    w_htoh4 = np.asarray(w_htoh4, dtype=np.float32)
    w_h4toh = np.asarray(w_h4toh, dtype=np.float32)

    B, d_model = inp.shape
    n_expert, d_ff, _ = w_htoh4.shape
    assert n_expert == NUM_EXPERT
    assert d_model == D_MODEL and d_ff == D_FF

    gi = gate_idx.astype(np.int64)
    order = np.argsort(gi, kind="stable")
    counts = np.bincount(gi, minlength=NUM_EXPERT)
    idx_split = np.split(order, np.cumsum(counts)[:-1])

    chunks = _capacity(counts.max())
    C = sum(chunks)

    scores_flat = gate_score.reshape(-1)

    nc = _build(chunks)

    in_maps = []
    for e in range(NUM_EXPERT):
        idx = idx_split[e]
        cnt = len(idx)
        xT = np.zeros((d_model, C), dtype=np.float32)
        if cnt:
            xT[:, :cnt] = (inp[idx] * scores_flat[idx][:, None]).T
        in_maps.append(_pack_expert(xT, w_htoh4[e], w_h4toh[e], chunks))

    from concourse import bass_utils
    res = bass_utils.run_bass_kernel_spmd(nc, in_maps,
                                          core_ids=list(range(N_CORES)))

    _last.update(nc=nc, in_maps=in_maps, res=res, chunks=chunks)

    y_full = np.empty((B, d_model), dtype=np.float32)
    for e in range(NUM_EXPERT):
        idx = idx_split[e]
        if len(idx) == 0:
            continue
        yt_h = np.asarray(res.results[e]["yt"], dtype=np.float32)  # (P,KO,C)
        yT = yt_h.transpose(1, 0, 2).reshape(d_model, C)
        y_full[idx] = yT[:, :len(idx)].T

    out = y_full[0::2] + y_full[1::2]
    return np.ascontiguousarray(out, dtype=np.float32)


# revision 70
# speedup vs baseline: 1.2514x; 1.0255x over previous
"""MoE (BruteForceMoELinear) Trainium2 kernel — fp8 DoubleRow expert-parallel.

Strategy: expert-parallel across 8 NeuronCores.  The host dispatches
token rows by `gate_idx` (stable sort), folds the per-row gate score
into the token vector (ReLU is positively homogeneous), pads each
expert's batch to capacity C = sum(chunks), and quantizes operands to
fp8-e4m3 with residual compensation:

  x  = xh + xl            (xh, xl fp8; xl = residual, fp8-exact range)
  W1 = W1h + W1l_s/S      (W1l_s = S*(W1-W1h) in fp8 — scaled so the
                           residual stays out of the subnormal range)
  W2 = W2h + W2l_s/S

Each GEMM runs 3 compensated fp8 DoubleRow matmul terms (K=256 per
instruction, 0.5 PE cycles/row — 2x the bf16 rate):

  h  = relu(xh@W1h + xl@W1h + (xh/S)@W1l_s)          [PSUM f32 accum]
  y  = hh@W2h + hl@W2h + (hh/S)@W2l_s                [PSUM f32 accum]

with hh = fp8(relu(p1)) on the scalar engine, hl = fp8(relu(p1)-hh) on
the vector engine (scalar_tensor_tensor), and hh/S on the gpsimd
engine (exact power-of-2 shift).  Measured end-to-end rel-err vs the
fp32 reference: ~3e-3 (tolerance 2e-2).

The host scatters per-expert outputs back to token order and sums the
top-k (=2) slots.
"""

import numpy as np
import ml_dtypes

BF16 = ml_dtypes.bfloat16
F8 = ml_dtypes.float8_e4m3   # matches mybir.dt.float8e4

NUM_EXPERT = 8
N_CORES = 8
P = 128
D_MODEL = 512
D_FF = 2048
KO = D_MODEL // P   # 4
FO = D_FF // P      # 16
S = 16.0            # residual scale (power of 2 — exact in fp8)

_CACHE = {}


def _build(chunks, repeat=1, n_warm=20, intro=None, split_last=0):
    """Compile the per-core program for capacity C = sum(chunks)."""
    if intro is None:
        # pair-step at which each chunk joins GEMM1 (chunk 0 leads, the
        # tail chunk last) — gives each x-chunk DMA time to land
        intro = tuple(0 if ch == 0 else (1 if ch == 1 else FO // 2 - 1)
                      for ch in range(len(chunks)))
    key = (tuple(chunks), repeat, n_warm, tuple(intro), split_last)
    if key in _CACHE:
        return _CACHE[key]

    import concourse.mybir as mybir
    import concourse.tile as tile
    from concourse import bacc

    f32 = mybir.dt.float32
    bf16 = mybir.dt.bfloat16
    fp8 = mybir.dt.float8e4
    DR = mybir.MatmulPerfMode.DoubleRow
    AOP = mybir.AluOpType
    C = sum(chunks)
    NCH = len(chunks)
    starts = [sum(chunks[:i]) for i in range(NCH)]

    nc = bacc.Bacc("TRN2", target_bir_lowering=False, debug=False,
                   num_devices=N_CORES)

    # merged operand tensors: one DMA feeds all three compensation
    # terms.  fp8 elements are 1 byte, so layouts keep the innermost
    # contiguous DRAM runs >= 512B (else the DMA pays a 2x latency
    # penalty): x is per-chunk (whole-tensor DMA), w1 is f-blocked.
    # xq* dim1: [xh ko0..3 | xl ko0..3 | xs ko0..3]
    # w1q: (P, f-block, [w1h ko0..3 | w1l ko0..3], 256)
    # w2q dim2: [w2h | w2l]
    NB = D_FF // 256
    xqs = [nc.dram_tensor(f"xq{ch}", (P, 3 * KO, chunks[ch]), fp8,
                          kind="ExternalInput") for ch in range(NCH)]
    w1q = nc.dram_tensor("w1q", (P, NB, 2 * KO, 256), fp8,
                         kind="ExternalInput")
    w2q = nc.dram_tensor("w2q", (P, KO, 2, FO, P), fp8,
                         kind="ExternalInput")
    yt = nc.dram_tensor("yt", (P, KO, C), bf16, kind="ExternalOutput")

    with tile.TileContext(nc) as tc:
        with tc.tile_pool(name="wpool", bufs=1) as wpool, \
             tc.tile_pool(name="xpool", bufs=1) as xpool, \
             tc.tile_pool(name="hpool", bufs=1) as hpool, \
             tc.tile_pool(name="ypool", bufs=8) as ypool, \
             tc.tile_pool(name="cpool", bufs=1) as cpool, \
             tc.tile_pool(name="ps1", bufs=4, space="PSUM") as ps1, \
             tc.tile_pool(name="ps2", bufs=4, space="PSUM") as ps2:

            bias0 = cpool.tile([P, 1], f32)
            nc.any.memset(bias0[:], 0.0)

            # PE warm-up through the DMA-priming window + Relu act-table
            # warm (reads SBUF — no coupling with the PE warm-up loop).
            # Wide (256-col) warm matmuls keep the p-state ramp alive to
            # the first real matmul with few instructions.
            warm = cpool.tile([P, 256], bf16)
            nc.vector.memset(warm[:], 0.5)
            warmh = cpool.tile([P, 64], bf16)
            nc.scalar.activation(warmh[:], warm[:, 0:64],
                                 mybir.ActivationFunctionType.Relu,
                                 bias=bias0[:])
            wps = ps2.tile([P, 2, 256], f32, name="warm", tag="p2")
            for _i in range(n_warm):
                nc.tensor.matmul(wps[:, 0, :], warm[:, 0:128], warm[:],
                                 start=True, stop=True)

            w1sb = wpool.tile([P, NB, 2 * KO, 256], fp8)
            w2sb = wpool.tile([P, KO, 2, FO, P], fp8)
            xsbs = [xpool.tile([P, 3 * KO, chunks[ch]], fp8, tag=f"x{ch}",
                               name=f"xsb{ch}") for ch in range(NCH)]

            # DMA emission order == consumption order.  256-f W1 pieces:
            # transfer time (~0.73us) just above the HWDGE descriptor-gen
            # serialization (~0.63us per DMA), and each piece exactly
            # feeds one fo-pair GEMM1 group.
            nc.sync.dma_start(xsbs[0][:], xqs[0].ap())
            for b in range(NB):
                nc.sync.dma_start(w1sb[:, b], w1q.ap()[:, b])
                if b == 1:
                    for ch in range(1, NCH):
                        nc.sync.dma_start(xsbs[ch][:], xqs[ch].ap())
            for db in range(KO):
                nc.sync.dma_start(w2sb[:, db], w2q.ap()[:, db])

            hhs = [hpool.tile([P, FO, chunks[ch]], fp8, tag=f"hh{ch}",
                              name=f"hh{ch}") for ch in range(NCH)]
            hls = [hpool.tile([P, FO, chunks[ch]], fp8, tag=f"hl{ch}",
                              name=f"hl{ch}") for ch in range(NCH)]

            def gemm1_pair(ch, fo, n_fo):
                # fo-QUAD group: one PSUM tile [P, n_fo, tn] (2 banks at
                # tn=256; each 1KB fo-slice stays inside a bank) holds
                # n_fo accumulation groups; ONE wide eviction per engine
                # amortizes the per-instruction PSUM access latency —
                # necessary because DVE's hl pass must keep up with the
                # fp8-DR PE rate.
                tn = chunks[ch]
                p1 = ps1.tile([P, n_fo, tn], f32, name="p1", tag="p1")
                terms = [(0, 0), (0, KO), (KO, 2 * KO)]
                for j in range(n_fo):
                    b, half = (fo + j) // 2, (fo + j) % 2
                    n = 0
                    for wo, xo in terms:
                        for t in range(KO // 2):
                            nc.tensor.matmul(
                                p1[:, j, :],
                                w1sb[:, b, wo + 2 * t:wo + 2 * t + 2,
                                     half * P:(half + 1) * P],
                                xsbs[ch][:, xo + 2 * t:xo + 2 * t + 2, :],
                                start=(n == 0),
                                stop=(n == 3 * (KO // 2) - 1),
                                perf_mode=DR)
                            n += 1
                nc.scalar.activation(
                    hhs[ch][:, fo:fo + n_fo, :], p1[:],
                    mybir.ActivationFunctionType.Relu, bias=bias0[:])
                # hl = fp8(max(p1,0) - hh) on DVE (gpsimd's
                # scalar_tensor_tensor fails the walrus verifier)
                nc.vector.scalar_tensor_tensor(
                    hls[ch][:, fo:fo + n_fo, :], p1[:], 0.0,
                    hhs[ch][:, fo:fo + n_fo, :], AOP.max, AOP.subtract)

            def gemm2(ch, db, t0, tn, csl, q=None):
                # all three terms accumulate S-scaled in ONE PSUM tile
                # (host stores W2h*S and W2l*S — exact fp8 exponent
                # shifts); the single 1/S rescale rides the activation
                # engine's built-in output scale at eviction.
                p2 = ps2.tile([P, tn], f32, name="p2", tag="p2")
                for n, (hsb, wi) in enumerate(
                        ((hhs[ch], 0), (hls[ch], 0), (hhs[ch], 1))):
                    for g in range(FO // 2):
                        nc.tensor.matmul(
                            p2[:],
                            w2sb[:, db, wi, 2 * g:2 * g + 2, :],
                            hsb[:, 2 * g:2 * g + 2, t0:t0 + tn],
                            start=(g == 0 and n == 0),
                            stop=(g == FO // 2 - 1 and n == 2),
                            perf_mode=DR)
                ysb = ypool.tile([P, tn], bf16, tag="y", name="ysb")
                nc.scalar.mul(ysb[:], p2[:], 1.0 / S)
                # stores ride the SP queue (the scalar sequencer must
                # stay free for the evictions); the second-to-last store
                # is diverted to gpsimd/SWDGE so the last two don't
                # serialize on the HWDGE device
                (q or nc.sync).dma_start(yt.ap()[:, db, csl], ysb[:])

            for _ in range(repeat):
                # phase 1: quad-step-major with staggered chunk intro
                # (chunk ch joins at quad-step intro[ch] and catches up
                # on its backlog).
                emitted = [0] * NCH
                for step in range(FO // 2):
                    for ch in range(NCH):
                        if intro[ch] <= step:
                            while emitted[ch] <= step:
                                gemm1_pair(ch, 2 * emitted[ch], 2)
                                emitted[ch] += 1
                for ch in range(NCH):
                    while emitted[ch] < FO // 2:
                        gemm1_pair(ch, 2 * emitted[ch], 2)
                        emitted[ch] += 1
                # phase 2: d-block-major, streamed out.  The final
                # d-block runs smallest chunks first so the last two
                # stores (the only ones with no compute left to hide
                # them) don't also queue on HWDGE behind a third.
                for db in range(KO):
                    order = list(range(NCH))
                    if db == KO - 1:
                        order.sort(key=lambda c: chunks[c])
                    for i, ch in enumerate(order):
                        q = (nc.gpsimd if db == KO - 1
                             and i == len(order) - 2 else None)
                        gemm2(ch, db, 0, chunks[ch],
                              slice(starts[ch], starts[ch] + chunks[ch]),
                              q=q)

    nc.compile()
    _CACHE[key] = nc
    return nc


def _capacity(max_count):
    """C >= max_count, multiple of 8; 256-token chunks (a 2-fo PSUM
    tile fills one 2KB bank exactly) plus a small remainder chunk."""
    maxc = max(int(max_count), 16)
    c = -(-maxc // 8) * 8
    chunks = []
    while c > 256:
        chunks.append(256)
        c -= 256
    chunks.append(c)
    return tuple(chunks)


def _q8(x):
    return np.asarray(x, dtype=np.float32).astype(F8)


def _pack_expert(xT, w1, w2, chunks):
    """Quantize + lay out one expert's operands.

    xT: (d_model, C) f32 (scores folded).  Returns the in_map with the
    per-chunk xq* plus w1q / w2q tensors (see _build for dim layouts).
    """
    C = xT.shape[1]
    assert sum(chunks) == C
    xh_f = _q8(xT)
    xl_f = _q8(xT - xh_f.astype(np.float32))
    xs_f = _q8(xh_f.astype(np.float32) / S)

    def pack_x(m):  # (d_model, C) -> (P, KO, C)
        return m.reshape(KO, P, C).transpose(1, 0, 2)

    xq = np.concatenate(
        [pack_x(xh_f), pack_x(xl_f), pack_x(xs_f)], axis=1)
    in_map = {}
    s0 = 0
    for ch, tn in enumerate(chunks):
        in_map[f"xq{ch}"] = np.ascontiguousarray(xq[:, :, s0:s0 + tn])
        s0 += tn

    # w1_h[p, ko, f] = W1[f, ko*128+p]; blocked (P, NB, 2*KO, 256)
    w1T = w1.T.reshape(KO, P, D_FF).transpose(1, 0, 2)
    w1h_f = _q8(w1T)
    w1l_f = _q8(S * (w1T - w1h_f.astype(np.float32)))
    w1cat = np.concatenate([w1h_f, w1l_f], axis=1)  # (P, 8, D_FF)
    NB = D_FF // 256
    in_map["w1q"] = np.ascontiguousarray(
        w1cat.reshape(P, 2 * KO, NB, 256).transpose(0, 2, 1, 3))

    # w2_h[p, db, fo, q] = W2[db*128+q, fo*128+p]; both W2 operands are
    # stored pre-scaled by S (exact fp8 exponent shift) so GEMM2's
    # three terms share one PSUM accumulation; eviction divides by S.
    w2T = w2.T.reshape(FO, P, KO, P).transpose(1, 2, 0, 3)
    w2h_f = _q8(w2T)
    w2hs_f = _q8(S * w2h_f.astype(np.float32))
    w2l_f = _q8(S * (w2T - w2h_f.astype(np.float32)))
    in_map["w2q"] = np.ascontiguousarray(
        np.stack([w2hs_f, w2l_f], axis=2))  # (P, KO, 2, FO, P)

    return in_map


_last = {}


def kernel(inp, gate_idx, gate_score, w_htoh4, w_h4toh):
    inp = np.ascontiguousarray(np.asarray(inp, dtype=np.float32))
    gate_idx = np.asarray(gate_idx)
    gate_score = np.asarray(gate_score, dtype=np.float32)
    w_htoh4 = np.asarray(w_htoh4, dtype=np.float32)
    w_h4toh = np.asarray(w_h4toh, dtype=np.float32)

    B, d_model = inp.shape
    n_expert, d_ff, _ = w_htoh4.shape
    assert n_expert == NUM_EXPERT
    assert d_model == D_MODEL and d_ff == D_FF

    gi = gate_idx.astype(np.int64)
    order = np.argsort(gi, kind="stable")
    counts = np.bincount(gi, minlength=NUM_EXPERT)
    idx_split = np.split(order, np.cumsum(counts)[:-1])

    chunks = _capacity(counts.max())
    C = sum(chunks)

    scores_flat = gate_score.reshape(-1)

    nc = _build(chunks)

    in_maps = []
    for e in range(NUM_EXPERT):
        idx = idx_split[e]
        cnt = len(idx)
        xT = np.zeros((d_model, C), dtype=np.float32)
        if cnt:
            xT[:, :cnt] = (inp[idx] * scores_flat[idx][:, None]).T
        in_maps.append(_pack_expert(xT, w_htoh4[e], w_h4toh[e], chunks))

    from concourse import bass_utils
    res = bass_utils.run_bass_kernel_spmd(nc, in_maps,
                                          core_ids=list(range(N_CORES)))

    _last.update(nc=nc, in_maps=in_maps, res=res, chunks=chunks)

    y_full = np.empty((B, d_model), dtype=np.float32)
    for e in range(NUM_EXPERT):
        idx = idx_split[e]
        if len(idx) == 0:
            continue
        yt_h = np.asarray(res.results[e]["yt"], dtype=np.float32)  # (P,KO,C)
        yT = yt_h.transpose(1, 0, 2).reshape(d_model, C)
        y_full[idx] = yT[:, :len(idx)].T

    out = y_full[0::2] + y_full[1::2]
    return np.ascontiguousarray(out, dtype=np.float32)


# revision 72
# speedup vs baseline: 1.2660x; 1.0117x over previous
"""MoE (BruteForceMoELinear) Trainium2 kernel — fp8 DoubleRow expert-parallel.

Strategy: expert-parallel across 8 NeuronCores.  The host dispatches
token rows by `gate_idx` (stable sort), folds the per-row gate score
into the token vector (ReLU is positively homogeneous), pads each
expert's batch to capacity C = sum(chunks), and quantizes operands to
fp8-e4m3 with residual compensation:

  x  = xh + xl            (xh, xl fp8; xl = residual, fp8-exact range)
  W1 = W1h + W1l_s/S      (W1l_s = S*(W1-W1h) in fp8 — scaled so the
                           residual stays out of the subnormal range)
  W2 = W2h + W2l_s/S

Each GEMM runs 3 compensated fp8 DoubleRow matmul terms (K=256 per
instruction, 0.5 PE cycles/row — 2x the bf16 rate):

  h  = relu(xh@W1h + xl@W1h + (xh/S)@W1l_s)          [PSUM f32 accum]
  y  = (hh@(S*W2h) + hl@(S*W2h) + hh@W2l_s) / S      [PSUM f32 accum]

with hh = fp8(relu(p1)) on the scalar engine and hl = fp8(relu(p1)-hh)
on the vector engine (scalar_tensor_tensor; its single allowed PSUM
input is p1).  Both W2 operands are stored pre-scaled by S (exact fp8
exponent shifts) so GEMM2's three terms share one PSUM accumulation;
the final 1/S rides the activation engine's output scale at eviction.
Measured end-to-end rel-err vs the fp32 reference: ~3e-3 (tolerance
2e-2); measured on-hardware kernel time ~31 us vs the 39.1 us bf32r
baseline.

The host scatters per-expert outputs back to token order and sums the
top-k (=2) slots.
"""

import numpy as np
import ml_dtypes

BF16 = ml_dtypes.bfloat16
F8 = ml_dtypes.float8_e4m3   # matches mybir.dt.float8e4

NUM_EXPERT = 8
N_CORES = 8
P = 128
D_MODEL = 512
D_FF = 2048
KO = D_MODEL // P   # 4
FO = D_FF // P      # 16
S = 16.0            # residual scale (power of 2 — exact in fp8)

_CACHE = {}


def _build(chunks, repeat=1, n_warm=17, intro=None, split_last=0):
    """Compile the per-core program for capacity C = sum(chunks)."""
    if intro is None:
        # pair-step at which each chunk joins GEMM1 (chunk 0 leads, the
        # tail chunk last) — gives each x-chunk DMA time to land
        intro = tuple(0 if ch == 0 else (1 if ch == 1 else FO // 2 - 1)
                      for ch in range(len(chunks)))
    key = (tuple(chunks), repeat, n_warm, tuple(intro), split_last)
    if key in _CACHE:
        return _CACHE[key]

    import concourse.mybir as mybir
    import concourse.tile as tile
    from concourse import bacc

    f32 = mybir.dt.float32
    bf16 = mybir.dt.bfloat16
    fp8 = mybir.dt.float8e4
    DR = mybir.MatmulPerfMode.DoubleRow
    AOP = mybir.AluOpType
    C = sum(chunks)
    NCH = len(chunks)
    starts = [sum(chunks[:i]) for i in range(NCH)]

    nc = bacc.Bacc("TRN2", target_bir_lowering=False, debug=False,
                   num_devices=N_CORES)

    # merged operand tensors: one DMA feeds all three compensation
    # terms.  fp8 elements are 1 byte, so layouts keep the innermost
    # contiguous DRAM runs >= 512B (else the DMA pays a 2x latency
    # penalty): x is per-chunk (whole-tensor DMA), w1 is f-blocked.
    # xq* dim1: [xh ko0..3 | xl ko0..3 | xs ko0..3]
    # w1q: (P, f-block, [w1h ko0..3 | w1l ko0..3], 256)
    # w2q dim2: [w2h | w2l]
    NB = D_FF // 256
    xqs = [nc.dram_tensor(f"xq{ch}", (P, 3 * KO, chunks[ch]), fp8,
                          kind="ExternalInput") for ch in range(NCH)]
    w1q = nc.dram_tensor("w1q", (P, NB, 2 * KO, 256), fp8,
                         kind="ExternalInput")
    w2q = nc.dram_tensor("w2q", (P, KO, 2, FO, P), fp8,
                         kind="ExternalInput")
    yt = nc.dram_tensor("yt", (P, KO, C), bf16, kind="ExternalOutput")

    with tile.TileContext(nc) as tc:
        with tc.tile_pool(name="wpool", bufs=1) as wpool, \
             tc.tile_pool(name="xpool", bufs=1) as xpool, \
             tc.tile_pool(name="hpool", bufs=1) as hpool, \
             tc.tile_pool(name="ypool", bufs=8) as ypool, \
             tc.tile_pool(name="cpool", bufs=1) as cpool, \
             tc.tile_pool(name="ps1", bufs=4, space="PSUM") as ps1, \
             tc.tile_pool(name="ps2", bufs=4, space="PSUM") as ps2:

            bias0 = cpool.tile([P, 1], f32)
            nc.any.memset(bias0[:], 0.0)

            # PE warm-up through the DMA-priming window + Relu act-table
            # warm (reads SBUF — no coupling with the PE warm-up loop).
            # Wide (256-col) warm matmuls keep the p-state ramp alive to
            # the first real matmul with few instructions.
            warm = cpool.tile([P, 256], bf16)
            nc.vector.memset(warm[:], 0.5)
            warmh = cpool.tile([P, 64], bf16)
            nc.scalar.activation(warmh[:], warm[:, 0:64],
                                 mybir.ActivationFunctionType.Relu,
                                 bias=bias0[:])
            wps = ps2.tile([P, 2, 256], f32, name="warm", tag="p2")
            for _i in range(n_warm):
                nc.tensor.matmul(wps[:, 0, :], warm[:, 0:128], warm[:],
                                 start=True, stop=True)

            w1sb = wpool.tile([P, NB, 2 * KO, 256], fp8)
            w2sb = wpool.tile([P, KO, 2, FO, P], fp8)
            xsbs = [xpool.tile([P, 3 * KO, chunks[ch]], fp8, tag=f"x{ch}",
                               name=f"xsb{ch}") for ch in range(NCH)]

            # DMA emission order == consumption order.  256-f W1 pieces:
            # transfer time (~0.73us) just above the HWDGE descriptor-gen
            # serialization (~0.63us per DMA), and each piece exactly
            # feeds one fo-pair GEMM1 group.
            nc.sync.dma_start(xsbs[0][:], xqs[0].ap())
            for b in range(NB):
                nc.sync.dma_start(w1sb[:, b], w1q.ap()[:, b])
                if b == 1:
                    for ch in range(1, NCH):
                        nc.sync.dma_start(xsbs[ch][:], xqs[ch].ap())
            for db in range(KO):
                nc.sync.dma_start(w2sb[:, db], w2q.ap()[:, db])

            hhs = [hpool.tile([P, FO, chunks[ch]], fp8, tag=f"hh{ch}",
                              name=f"hh{ch}") for ch in range(NCH)]
            hls = [hpool.tile([P, FO, chunks[ch]], fp8, tag=f"hl{ch}",
                              name=f"hl{ch}") for ch in range(NCH)]

            def gemm1_pair(ch, fo, n_fo):
                # fo-QUAD group: one PSUM tile [P, n_fo, tn] (2 banks at
                # tn=256; each 1KB fo-slice stays inside a bank) holds
                # n_fo accumulation groups; ONE wide eviction per engine
                # amortizes the per-instruction PSUM access latency —
                # necessary because DVE's hl pass must keep up with the
                # fp8-DR PE rate.
                tn = chunks[ch]
                p1 = ps1.tile([P, n_fo, tn], f32, name="p1", tag="p1")
                terms = [(0, 0), (0, KO), (KO, 2 * KO)]
                for j in range(n_fo):
                    b, half = (fo + j) // 2, (fo + j) % 2
                    n = 0
                    for wo, xo in terms:
                        for t in range(KO // 2):
                            nc.tensor.matmul(
                                p1[:, j, :],
                                w1sb[:, b, wo + 2 * t:wo + 2 * t + 2,
                                     half * P:(half + 1) * P],
                                xsbs[ch][:, xo + 2 * t:xo + 2 * t + 2, :],
                                start=(n == 0),
                                stop=(n == 3 * (KO // 2) - 1),
                                perf_mode=DR)
                            n += 1
                nc.scalar.activation(
                    hhs[ch][:, fo:fo + n_fo, :], p1[:],
                    mybir.ActivationFunctionType.Relu, bias=bias0[:])
                # hl = fp8(max(p1,0) - hh) on DVE (gpsimd's
                # scalar_tensor_tensor fails the walrus verifier)
                nc.vector.scalar_tensor_tensor(
                    hls[ch][:, fo:fo + n_fo, :], p1[:], 0.0,
                    hhs[ch][:, fo:fo + n_fo, :], AOP.max, AOP.subtract)

            def gemm2(ch, db, t0, tn, csl, q=None):
                # all three terms accumulate S-scaled in ONE PSUM tile
                # (host stores W2h*S and W2l*S — exact fp8 exponent
                # shifts); the single 1/S rescale rides the activation
                # engine's built-in output scale at eviction.
                p2 = ps2.tile([P, tn], f32, name="p2", tag="p2")
                for n, (hsb, wi) in enumerate(
                        ((hhs[ch], 0), (hls[ch], 0), (hhs[ch], 1))):
                    for g in range(FO // 2):
                        nc.tensor.matmul(
                            p2[:],
                            w2sb[:, db, wi, 2 * g:2 * g + 2, :],
                            hsb[:, 2 * g:2 * g + 2, t0:t0 + tn],
                            start=(g == 0 and n == 0),
                            stop=(g == FO // 2 - 1 and n == 2),
                            perf_mode=DR)
                ysb = ypool.tile([P, tn], bf16, tag="y", name="ysb")
                nc.scalar.mul(ysb[:], p2[:], 1.0 / S)
                # stores ride the SP queue (the scalar sequencer must
                # stay free for the evictions); the second-to-last store
                # is diverted to gpsimd/SWDGE so the last two don't
                # serialize on the HWDGE device
                (q or nc.sync).dma_start(yt.ap()[:, db, csl], ysb[:])

            for _ in range(repeat):
                # phase 1: quad-step-major with staggered chunk intro
                # (chunk ch joins at quad-step intro[ch] and catches up
                # on its backlog).
                emitted = [0] * NCH
                for step in range(FO // 2):
                    for ch in range(NCH):
                        if intro[ch] <= step:
                            while emitted[ch] <= step:
                                gemm1_pair(ch, 2 * emitted[ch], 2)
                                emitted[ch] += 1
                for ch in range(NCH):
                    while emitted[ch] < FO // 2:
                        gemm1_pair(ch, 2 * emitted[ch], 2)
                        emitted[ch] += 1
                # phase 2: d-block-major, streamed out.  The final
                # d-block runs smallest chunks first so the last two
                # stores (the only ones with no compute left to hide
                # them) don't also queue on HWDGE behind a third.
                for db in range(KO):
                    order = list(range(NCH))
                    if db == KO - 1:
                        order.sort(key=lambda c: chunks[c])
                    for i, ch in enumerate(order):
                        q = (nc.gpsimd if db == KO - 1
                             and i == len(order) - 2 else None)
                        gemm2(ch, db, 0, chunks[ch],
                              slice(starts[ch], starts[ch] + chunks[ch]),
                              q=q)

    nc.compile()
    _CACHE[key] = nc
    return nc


def _capacity(max_count):
    """C >= max_count, multiple of 8; 256-token chunks (a 2-fo PSUM
    tile fills one 2KB bank exactly) plus a small remainder chunk."""
    maxc = max(int(max_count), 16)
    c = -(-maxc // 8) * 8
    chunks = []
    while c > 256:
        chunks.append(256)
        c -= 256
    chunks.append(c)
    return tuple(chunks)


def _q8(x):
    return np.asarray(x, dtype=np.float32).astype(F8)


def _pack_expert(xT, w1, w2, chunks):
    """Quantize + lay out one expert's operands.

    xT: (d_model, C) f32 (scores folded).  Returns the in_map with the
    per-chunk xq* plus w1q / w2q tensors (see _build for dim layouts).
    """
    C = xT.shape[1]
    assert sum(chunks) == C
    xh_f = _q8(xT)
    xl_f = _q8(xT - xh_f.astype(np.float32))
    xs_f = _q8(xh_f.astype(np.float32) / S)

    def pack_x(m):  # (d_model, C) -> (P, KO, C)
        return m.reshape(KO, P, C).transpose(1, 0, 2)

    xq = np.concatenate(
        [pack_x(xh_f), pack_x(xl_f), pack_x(xs_f)], axis=1)
    in_map = {}
    s0 = 0
    for ch, tn in enumerate(chunks):
        in_map[f"xq{ch}"] = np.ascontiguousarray(xq[:, :, s0:s0 + tn])
        s0 += tn

    # w1_h[p, ko, f] = W1[f, ko*128+p]; blocked (P, NB, 2*KO, 256)
    w1T = w1.T.reshape(KO, P, D_FF).transpose(1, 0, 2)
    w1h_f = _q8(w1T)
    w1l_f = _q8(S * (w1T - w1h_f.astype(np.float32)))
    w1cat = np.concatenate([w1h_f, w1l_f], axis=1)  # (P, 8, D_FF)
    NB = D_FF // 256
    in_map["w1q"] = np.ascontiguousarray(
        w1cat.reshape(P, 2 * KO, NB, 256).transpose(0, 2, 1, 3))

    # w2_h[p, db, fo, q] = W2[db*128+q, fo*128+p]; both W2 operands are
    # stored pre-scaled by S (exact fp8 exponent shift) so GEMM2's
    # three terms share one PSUM accumulation; eviction divides by S.
    w2T = w2.T.reshape(FO, P, KO, P).transpose(1, 2, 0, 3)
    w2h_f = _q8(w2T)
    w2hs_f = _q8(S * w2h_f.astype(np.float32))
    w2l_f = _q8(S * (w2T - w2h_f.astype(np.float32)))
    in_map["w2q"] = np.ascontiguousarray(
        np.stack([w2hs_f, w2l_f], axis=2))  # (P, KO, 2, FO, P)

    return in_map


_last = {}


def kernel(inp, gate_idx, gate_score, w_htoh4, w_h4toh):
    inp = np.ascontiguousarray(np.asarray(inp, dtype=np.float32))
    gate_idx = np.asarray(gate_idx)
    gate_score = np.asarray(gate_score, dtype=np.float32)
    w_htoh4 = np.asarray(w_htoh4, dtype=np.float32)
    w_h4toh = np.asarray(w_h4toh, dtype=np.float32)

    B, d_model = inp.shape
    n_expert, d_ff, _ = w_htoh4.shape
    assert n_expert == NUM_EXPERT
    assert d_model == D_MODEL and d_ff == D_FF

    gi = gate_idx.astype(np.int64)
    order = np.argsort(gi, kind="stable")
    counts = np.bincount(gi, minlength=NUM_EXPERT)
    idx_split = np.split(order, np.cumsum(counts)[:-1])

    chunks = _capacity(counts.max())
    C = sum(chunks)

    scores_flat = gate_score.reshape(-1)

    nc = _build(chunks)

    in_maps = []
    for e in range(NUM_EXPERT):
        idx = idx_split[e]
        cnt = len(idx)
        xT = np.zeros((d_model, C), dtype=np.float32)
        if cnt:
            xT[:, :cnt] = (inp[idx] * scores_flat[idx][:, None]).T
        in_maps.append(_pack_expert(xT, w_htoh4[e], w_h4toh[e], chunks))

    from concourse import bass_utils
    res = bass_utils.run_bass_kernel_spmd(nc, in_maps,
                                          core_ids=list(range(N_CORES)))

    _last.update(nc=nc, in_maps=in_maps, res=res, chunks=chunks)

    y_full = np.empty((B, d_model), dtype=np.float32)
    for e in range(NUM_EXPERT):
        idx = idx_split[e]
        if len(idx) == 0:
            continue
        yt_h = np.asarray(res.results[e]["yt"], dtype=np.float32)  # (P,KO,C)
        yT = yt_h.transpose(1, 0, 2).reshape(d_model, C)
        y_full[idx] = yT[:, :len(idx)].T

    out = y_full[0::2] + y_full[1::2]
    return np.ascontiguousarray(out, dtype=np.float32)


# revision 80
# speedup vs baseline: 1.2692x; 1.0025x over previous
"""MoE (BruteForceMoELinear) Trainium2 kernel — fp8 DoubleRow expert-parallel.

Strategy: expert-parallel across 8 NeuronCores.  The host dispatches
token rows by `gate_idx` (stable sort), folds the per-row gate score
into the token vector (ReLU is positively homogeneous), pads each
expert's batch to capacity C = sum(chunks), and quantizes operands to
fp8-e4m3 with residual compensation:

  x  = xh + xl            (xh, xl fp8; xl = residual, fp8-exact range)
  W1 = W1h + W1l_s/S      (W1l_s = S*(W1-W1h) in fp8 — scaled so the
                           residual stays out of the subnormal range)
  W2 = W2h + W2l_s/S

Each GEMM runs 3 compensated fp8 DoubleRow matmul terms (K=256 per
instruction, 0.5 PE cycles/row — 2x the bf16 rate):

  h  = relu(xh@W1h + xl@W1h + (xh/S)@W1l_s)          [PSUM f32 accum]
  y  = (hh@(S*W2h) + hl@(S*W2h) + hh@W2l_s) / S      [PSUM f32 accum]

with hh = fp8(relu(p1)) on the scalar engine and hl = fp8(relu(p1)-hh)
on the vector engine (scalar_tensor_tensor; its single allowed PSUM
input is p1).  Both W2 operands are stored pre-scaled by S (exact fp8
exponent shifts) so GEMM2's three terms share one PSUM accumulation;
the final 1/S rides the activation engine's output scale at eviction.
Measured end-to-end rel-err vs the fp32 reference: ~3e-3 (tolerance
2e-2); measured on-hardware kernel time ~31 us vs the 39.1 us bf32r
baseline.

The host scatters per-expert outputs back to token order and sums the
top-k (=2) slots.
"""

import numpy as np
import ml_dtypes

BF16 = ml_dtypes.bfloat16
F8 = ml_dtypes.float8_e4m3   # matches mybir.dt.float8e4

NUM_EXPERT = 8
N_CORES = 8
P = 128
D_MODEL = 512
D_FF = 2048
KO = D_MODEL // P   # 4
FO = D_FF // P      # 16
S = 16.0            # residual scale (power of 2 — exact in fp8)

_CACHE = {}


def _build(chunks, repeat=1, n_warm=17, intro=None, split_last=0):
    """Compile the per-core program for capacity C = sum(chunks)."""
    if intro is None:
        # pair-step at which each chunk joins GEMM1 (chunk 0 leads, the
        # tail chunk last) — gives each x-chunk DMA time to land
        intro = tuple(0 if ch == 0 else (1 if ch == 1 else FO // 2 - 1)
                      for ch in range(len(chunks)))
    key = (tuple(chunks), repeat, n_warm, tuple(intro), split_last)
    if key in _CACHE:
        return _CACHE[key]

    import concourse.mybir as mybir
    import concourse.tile as tile
    from concourse import bacc

    f32 = mybir.dt.float32
    bf16 = mybir.dt.bfloat16
    fp8 = mybir.dt.float8e4
    DR = mybir.MatmulPerfMode.DoubleRow
    AOP = mybir.AluOpType
    C = sum(chunks)
    NCH = len(chunks)
    starts = [sum(chunks[:i]) for i in range(NCH)]

    nc = bacc.Bacc("TRN2", target_bir_lowering=False, debug=False,
                   num_devices=N_CORES)

    # merged operand tensors: one DMA feeds all three compensation
    # terms.  fp8 elements are 1 byte, so layouts keep the innermost
    # contiguous DRAM runs >= 512B (else the DMA pays a 2x latency
    # penalty): x is per-chunk (whole-tensor DMA), w1 is f-blocked.
    # xq* dim1: [xh ko0..3 | xl ko0..3 | xs ko0..3]
    # w1q: (P, f-block, [w1h ko0..3 | w1l ko0..3], 256)
    # w2q dim2: [w2h | w2l]
    NB = D_FF // 256
    xqs = [nc.dram_tensor(f"xq{ch}", (P, 3 * KO, chunks[ch]), fp8,
                          kind="ExternalInput") for ch in range(NCH)]
    w1q = nc.dram_tensor("w1q", (P, NB, 2 * KO, 256), fp8,
                         kind="ExternalInput")
    w2q = nc.dram_tensor("w2q", (P, KO, 2, FO, P), fp8,
                         kind="ExternalInput")
    yt = nc.dram_tensor("yt", (P, KO, C), bf16, kind="ExternalOutput")

    with tile.TileContext(nc) as tc:
        with tc.tile_pool(name="wpool", bufs=1) as wpool, \
             tc.tile_pool(name="xpool", bufs=1) as xpool, \
             tc.tile_pool(name="hpool", bufs=1) as hpool, \
             tc.tile_pool(name="ypool", bufs=13) as ypool, \
             tc.tile_pool(name="cpool", bufs=1) as cpool, \
             tc.tile_pool(name="ps1", bufs=4, space="PSUM") as ps1, \
             tc.tile_pool(name="ps2", bufs=4, space="PSUM") as ps2:

            bias0 = cpool.tile([P, 1], f32)
            nc.any.memset(bias0[:], 0.0)

            # PE warm-up through the DMA-priming window + Relu act-table
            # warm (reads SBUF — no coupling with the PE warm-up loop).
            # Wide (256-col) warm matmuls keep the p-state ramp alive to
            # the first real matmul with few instructions.
            warm = cpool.tile([P, 256], bf16)
            nc.vector.memset(warm[:], 0.5)
            warmh = cpool.tile([P, 64], bf16)
            nc.scalar.activation(warmh[:], warm[:, 0:64],
                                 mybir.ActivationFunctionType.Relu,
                                 bias=bias0[:])
            wps = ps2.tile([P, 2, 256], f32, name="warm", tag="p2")
            for _i in range(n_warm):
                nc.tensor.matmul(wps[:, 0, :], warm[:, 0:128], warm[:],
                                 start=True, stop=True)

            w1sb = wpool.tile([P, NB, 2 * KO, 256], fp8)
            w2sb = wpool.tile([P, KO, 2, FO, P], fp8)
            xsbs = [xpool.tile([P, 3 * KO, chunks[ch]], fp8, tag=f"x{ch}",
                               name=f"xsb{ch}") for ch in range(NCH)]

            # DMA emission order == consumption order.  256-f W1 pieces:
            # transfer time (~0.73us) just above the HWDGE descriptor-gen
            # serialization (~0.63us per DMA), and each piece exactly
            # feeds one fo-pair GEMM1 group.
            nc.sync.dma_start(xsbs[0][:], xqs[0].ap())
            for b in range(NB):
                nc.sync.dma_start(w1sb[:, b], w1q.ap()[:, b])
                if b == 1:
                    for ch in range(1, NCH):
                        nc.sync.dma_start(xsbs[ch][:], xqs[ch].ap())
            for db in range(KO):
                nc.sync.dma_start(w2sb[:, db], w2q.ap()[:, db])

            hhs = [hpool.tile([P, FO, chunks[ch]], fp8, tag=f"hh{ch}",
                              name=f"hh{ch}") for ch in range(NCH)]
            hls = [hpool.tile([P, FO, chunks[ch]], fp8, tag=f"hl{ch}",
                              name=f"hl{ch}") for ch in range(NCH)]

            def gemm1_pair(ch, fo, n_fo):
                # fo-QUAD group: one PSUM tile [P, n_fo, tn] (2 banks at
                # tn=256; each 1KB fo-slice stays inside a bank) holds
                # n_fo accumulation groups; ONE wide eviction per engine
                # amortizes the per-instruction PSUM access latency —
                # necessary because DVE's hl pass must keep up with the
                # fp8-DR PE rate.
                tn = chunks[ch]
                p1 = ps1.tile([P, n_fo, tn], f32, name="p1", tag="p1")
                terms = [(0, 0), (0, KO), (KO, 2 * KO)]
                for j in range(n_fo):
                    b, half = (fo + j) // 2, (fo + j) % 2
                    n = 0
                    for wo, xo in terms:
                        for t in range(KO // 2):
                            nc.tensor.matmul(
                                p1[:, j, :],
                                w1sb[:, b, wo + 2 * t:wo + 2 * t + 2,
                                     half * P:(half + 1) * P],
                                xsbs[ch][:, xo + 2 * t:xo + 2 * t + 2, :],
                                start=(n == 0),
                                stop=(n == 3 * (KO // 2) - 1),
                                perf_mode=DR)
                            n += 1
                nc.scalar.activation(
                    hhs[ch][:, fo:fo + n_fo, :], p1[:],
                    mybir.ActivationFunctionType.Relu, bias=bias0[:])
                # hl = fp8(max(p1,0) - hh) on DVE (gpsimd's
                # scalar_tensor_tensor fails the walrus verifier)
                nc.vector.scalar_tensor_tensor(
                    hls[ch][:, fo:fo + n_fo, :], p1[:], 0.0,
                    hhs[ch][:, fo:fo + n_fo, :], AOP.max, AOP.subtract)

            def gemm2(ch, db, t0, tn, csl, q=None):
                # all three terms accumulate S-scaled in ONE PSUM tile
                # (host stores W2h*S and W2l*S — exact fp8 exponent
                # shifts); the single 1/S rescale rides the activation
                # engine's built-in output scale at eviction.
                p2 = ps2.tile([P, tn], f32, name="p2", tag="p2")
                for n, (hsb, wi) in enumerate(
                        ((hhs[ch], 0), (hls[ch], 0), (hhs[ch], 1))):
                    for g in range(FO // 2):
                        nc.tensor.matmul(
                            p2[:],
                            w2sb[:, db, wi, 2 * g:2 * g + 2, :],
                            hsb[:, 2 * g:2 * g + 2, t0:t0 + tn],
                            start=(g == 0 and n == 0),
                            stop=(g == FO // 2 - 1 and n == 2),
                            perf_mode=DR)
                ysb = ypool.tile([P, tn], bf16, tag="y", name="ysb")
                nc.scalar.mul(ysb[:], p2[:], 1.0 / S)
                # stores ride the SP queue (the scalar sequencer must
                # stay free for the evictions); the second-to-last store
                # is diverted to gpsimd/SWDGE so the last two don't
                # serialize on the HWDGE device
                (q or nc.sync).dma_start(yt.ap()[:, db, csl], ysb[:])

            for _ in range(repeat):
                # phase 1: quad-step-major with staggered chunk intro
                # (chunk ch joins at quad-step intro[ch] and catches up
                # on its backlog).
                emitted = [0] * NCH
                for step in range(FO // 2):
                    for ch in range(NCH):
                        if intro[ch] <= step:
                            while emitted[ch] <= step:
                                gemm1_pair(ch, 2 * emitted[ch], 2)
                                emitted[ch] += 1
                for ch in range(NCH):
                    while emitted[ch] < FO // 2:
                        gemm1_pair(ch, 2 * emitted[ch], 2)
                        emitted[ch] += 1
                # phase 2: d-block-major, streamed out.  The final
                # d-block runs smallest chunks first so the last two
                # stores (the only ones with no compute left to hide
                # them) don't also queue on HWDGE behind a third.
                for db in range(KO):
                    order = list(range(NCH))
                    if db == KO - 1:
                        order.sort(key=lambda c: chunks[c])
                    for i, ch in enumerate(order):
                        tn = chunks[ch]
                        s0 = starts[ch]
                        # second-to-last store rides gpsimd/SWDGE so the
                        # last two stores don't serialize on HWDGE
                        q = (nc.gpsimd if db == KO - 1
                             and i == len(order) - 2 else None)
                        gemm2(ch, db, 0, tn, slice(s0, s0 + tn), q=q)

    nc.compile()
    _CACHE[key] = nc
    return nc


def _capacity(max_count):
    """C >= max_count, multiple of 8; 256-token chunks (a 2-fo PSUM
    tile fills one 2KB bank exactly) plus a small remainder chunk."""
    maxc = max(int(max_count), 16)
    c = -(-maxc // 8) * 8
    chunks = []
    while c > 256:
        chunks.append(256)
        c -= 256
    chunks.append(c)
    return tuple(chunks)


def _q8(x):
    return np.asarray(x, dtype=np.float32).astype(F8)


def _pack_expert(xT, w1, w2, chunks):
    """Quantize + lay out one expert's operands.

    xT: (d_model, C) f32 (scores folded).  Returns the in_map with the
    per-chunk xq* plus w1q / w2q tensors (see _build for dim layouts).
    """
    C = xT.shape[1]
    assert sum(chunks) == C
    xh_f = _q8(xT)
    xl_f = _q8(xT - xh_f.astype(np.float32))
    xs_f = _q8(xh_f.astype(np.float32) / S)

    def pack_x(m):  # (d_model, C) -> (P, KO, C)
        return m.reshape(KO, P, C).transpose(1, 0, 2)

    xq = np.concatenate(
        [pack_x(xh_f), pack_x(xl_f), pack_x(xs_f)], axis=1)
    in_map = {}
    s0 = 0
    for ch, tn in enumerate(chunks):
        in_map[f"xq{ch}"] = np.ascontiguousarray(xq[:, :, s0:s0 + tn])
        s0 += tn

    # w1_h[p, ko, f] = W1[f, ko*128+p]; blocked (P, NB, 2*KO, 256)
    w1T = w1.T.reshape(KO, P, D_FF).transpose(1, 0, 2)
    w1h_f = _q8(w1T)
    w1l_f = _q8(S * (w1T - w1h_f.astype(np.float32)))
    w1cat = np.concatenate([w1h_f, w1l_f], axis=1)  # (P, 8, D_FF)
    NB = D_FF // 256
    in_map["w1q"] = np.ascontiguousarray(
        w1cat.reshape(P, 2 * KO, NB, 256).transpose(0, 2, 1, 3))

    # w2_h[p, db, fo, q] = W2[db*128+q, fo*128+p]; both W2 operands are
    # stored pre-scaled by S (exact fp8 exponent shift) so GEMM2's
    # three terms share one PSUM accumulation; eviction divides by S.
    w2T = w2.T.reshape(FO, P, KO, P).transpose(1, 2, 0, 3)
    w2h_f = _q8(w2T)
    w2hs_f = _q8(S * w2h_f.astype(np.float32))
    w2l_f = _q8(S * (w2T - w2h_f.astype(np.float32)))
    in_map["w2q"] = np.ascontiguousarray(
        np.stack([w2hs_f, w2l_f], axis=2))  # (P, KO, 2, FO, P)

    return in_map


_last = {}


def kernel(inp, gate_idx, gate_score, w_htoh4, w_h4toh):
    inp = np.ascontiguousarray(np.asarray(inp, dtype=np.float32))
    gate_idx = np.asarray(gate_idx)
    gate_score = np.asarray(gate_score, dtype=np.float32)
    w_htoh4 = np.asarray(w_htoh4, dtype=np.float32)
    w_h4toh = np.asarray(w_h4toh, dtype=np.float32)

    B, d_model = inp.shape
    n_expert, d_ff, _ = w_htoh4.shape
    assert n_expert == NUM_EXPERT
    assert d_model == D_MODEL and d_ff == D_FF

    gi = gate_idx.astype(np.int64)
    order = np.argsort(gi, kind="stable")
    counts = np.bincount(gi, minlength=NUM_EXPERT)
    idx_split = np.split(order, np.cumsum(counts)[:-1])

    chunks = _capacity(counts.max())
    C = sum(chunks)

    scores_flat = gate_score.reshape(-1)

    nc = _build(chunks)

    in_maps = []
    for e in range(NUM_EXPERT):
        idx = idx_split[e]
        cnt = len(idx)
        xT = np.zeros((d_model, C), dtype=np.float32)
        if cnt:
            xT[:, :cnt] = (inp[idx] * scores_flat[idx][:, None]).T
        in_maps.append(_pack_expert(xT, w_htoh4[e], w_h4toh[e], chunks))

    from concourse import bass_utils
    res = bass_utils.run_bass_kernel_spmd(nc, in_maps,
                                          core_ids=list(range(N_CORES)))

    _last.update(nc=nc, in_maps=in_maps, res=res, chunks=chunks)

    y_full = np.empty((B, d_model), dtype=np.float32)
    for e in range(NUM_EXPERT):
        idx = idx_split[e]
        if len(idx) == 0:
            continue
        yt_h = np.asarray(res.results[e]["yt"], dtype=np.float32)  # (P,KO,C)
        yT = yt_h.transpose(1, 0, 2).reshape(d_model, C)
        y_full[idx] = yT[:, :len(idx)].T

    out = y_full[0::2] + y_full[1::2]
    return np.ascontiguousarray(out, dtype=np.float32)


# revision 83
# speedup vs baseline: 1.2872x; 1.0142x over previous
"""MoE (BruteForceMoELinear) Trainium2 kernel — fp8 DoubleRow expert-parallel.

Strategy: expert-parallel across 8 NeuronCores.  The host dispatches
token rows by `gate_idx` (stable sort), folds the per-row gate score
into the token vector (ReLU is positively homogeneous), pads each
expert's batch to capacity C = sum(chunks), and quantizes operands to
fp8-e4m3 with residual compensation:

  x  = xh + xl            (xh, xl fp8; xl = residual, fp8-exact range)
  W1 = W1h + W1l_s/S      (W1l_s = S*(W1-W1h) in fp8 — scaled so the
                           residual stays out of the subnormal range)
  W2 = W2h + W2l_s/S

Each GEMM runs 3 compensated fp8 DoubleRow matmul terms (K=256 per
instruction, 0.5 PE cycles/row — 2x the bf16 rate):

  h  = relu(xh@W1h + xl@W1h + (xh/S)@W1l_s)          [PSUM f32 accum]
  y  = (hh@(S*W2h) + hl@(S*W2h) + hh@W2l_s) / S      [PSUM f32 accum]

with hh = fp8(relu(p1)) on the scalar engine and hl = fp8(relu(p1)-hh)
on the vector engine (scalar_tensor_tensor; its single allowed PSUM
input is p1).  Both W2 operands are stored pre-scaled by S (exact fp8
exponent shifts) so GEMM2's three terms share one PSUM accumulation;
the final 1/S rides the activation engine's output scale at eviction.
Measured end-to-end rel-err vs the fp32 reference: ~3e-3 (tolerance
2e-2); measured on-hardware kernel time ~31 us vs the 39.1 us bf32r
baseline.

The host scatters per-expert outputs back to token order and sums the
top-k (=2) slots.
"""

import numpy as np
import ml_dtypes

BF16 = ml_dtypes.bfloat16
F8 = ml_dtypes.float8_e4m3   # matches mybir.dt.float8e4

NUM_EXPERT = 8
N_CORES = 8
P = 128
D_MODEL = 512
D_FF = 2048
KO = D_MODEL // P   # 4
FO = D_FF // P      # 16
S = 16.0            # residual scale (power of 2 — exact in fp8)

_CACHE = {}


def _build(chunks, repeat=1, n_warm=17, intro=None, split_last=0):
    """Compile the per-core program for capacity C = sum(chunks)."""
    if intro is None:
        # pair-step at which each chunk joins GEMM1 (chunk 0 leads, the
        # tail chunk last) — gives each x-chunk DMA time to land
        intro = tuple(0 if ch == 0 else (1 if ch == 1 else FO // 2 - 1)
                      for ch in range(len(chunks)))
    key = (tuple(chunks), repeat, n_warm, tuple(intro), split_last)
    if key in _CACHE:
        return _CACHE[key]

    import concourse.mybir as mybir
    import concourse.tile as tile
    from concourse import bacc

    f32 = mybir.dt.float32
    bf16 = mybir.dt.bfloat16
    fp8 = mybir.dt.float8e4
    DR = mybir.MatmulPerfMode.DoubleRow
    AOP = mybir.AluOpType
    C = sum(chunks)
    NCH = len(chunks)
    starts = [sum(chunks[:i]) for i in range(NCH)]

    nc = bacc.Bacc("TRN2", target_bir_lowering=False, debug=False,
                   num_devices=N_CORES)

    # merged operand tensors: one DMA feeds all three compensation
    # terms.  fp8 elements are 1 byte, so layouts keep the innermost
    # contiguous DRAM runs >= 512B (else the DMA pays a 2x latency
    # penalty): x is per-chunk (whole-tensor DMA), w1 is f-blocked.
    # xq* dim1: [xh ko0..3 | xl ko0..3 | xs ko0..3]
    # w1q: (P, f-block, [w1h ko0..3 | w1l ko0..3], 256)
    # w2q dim2: [w2h | w2l]
    NB = D_FF // 256
    xqs = [nc.dram_tensor(f"xq{ch}", (P, 3 * KO, chunks[ch]), fp8,
                          kind="ExternalInput") for ch in range(NCH)]
    w1q = nc.dram_tensor("w1q", (P, NB, 2 * KO, 256), fp8,
                         kind="ExternalInput")
    w2q = nc.dram_tensor("w2q", (P, KO, 2, FO, P), fp8,
                         kind="ExternalInput")
    yt = nc.dram_tensor("yt", (P, KO, C), bf16, kind="ExternalOutput")

    with tile.TileContext(nc) as tc:
        with tc.tile_pool(name="wpool", bufs=1) as wpool, \
             tc.tile_pool(name="xpool", bufs=1) as xpool, \
             tc.tile_pool(name="hpool", bufs=1) as hpool, \
             tc.tile_pool(name="ypool", bufs=13) as ypool, \
             tc.tile_pool(name="cpool", bufs=1) as cpool, \
             tc.tile_pool(name="ps1", bufs=4, space="PSUM") as ps1, \
             tc.tile_pool(name="ps2", bufs=4, space="PSUM") as ps2:

            bias0 = cpool.tile([P, 1], f32)
            nc.any.memset(bias0[:], 0.0)

            # PE warm-up through the DMA-priming window + Relu act-table
            # warm (reads SBUF — no coupling with the PE warm-up loop).
            # Wide (256-col) warm matmuls keep the p-state ramp alive to
            # the first real matmul with few instructions.
            warm = cpool.tile([P, 256], bf16)
            nc.vector.memset(warm[:], 0.5)
            warmh = cpool.tile([P, 64], bf16)
            nc.scalar.activation(warmh[:], warm[:, 0:64],
                                 mybir.ActivationFunctionType.Relu,
                                 bias=bias0[:])
            wps = ps2.tile([P, 2, 256], f32, name="warm", tag="p2")
            for _i in range(n_warm):
                nc.tensor.matmul(wps[:, 0, :], warm[:, 0:128], warm[:],
                                 start=True, stop=True)

            w1sb = wpool.tile([P, NB, 2 * KO, 256], fp8)
            w2sb = wpool.tile([P, KO, 2, FO, P], fp8)
            xsbs = [xpool.tile([P, 3 * KO, chunks[ch]], fp8, tag=f"x{ch}",
                               name=f"xsb{ch}") for ch in range(NCH)]

            # DMA emission order == consumption order.  256-f W1 pieces:
            # transfer time (~0.73us) just above the HWDGE descriptor-gen
            # serialization (~0.63us per DMA), and each piece exactly
            # feeds one fo-pair GEMM1 group.
            nc.sync.dma_start(xsbs[0][:], xqs[0].ap())
            for b in range(NB):
                nc.sync.dma_start(w1sb[:, b], w1q.ap()[:, b])
                if b == 1:
                    for ch in range(1, NCH):
                        nc.sync.dma_start(xsbs[ch][:], xqs[ch].ap())
            for db in range(KO):
                nc.sync.dma_start(w2sb[:, db], w2q.ap()[:, db])

            hhs = [hpool.tile([P, FO, chunks[ch]], fp8, tag=f"hh{ch}",
                              name=f"hh{ch}") for ch in range(NCH)]
            hls = [hpool.tile([P, FO, chunks[ch]], fp8, tag=f"hl{ch}",
                              name=f"hl{ch}") for ch in range(NCH)]

            def gemm1_pair(ch, fo, n_fo):
                # fo-QUAD group: one PSUM tile [P, n_fo, tn] (2 banks at
                # tn=256; each 1KB fo-slice stays inside a bank) holds
                # n_fo accumulation groups; ONE wide eviction per engine
                # amortizes the per-instruction PSUM access latency —
                # necessary because DVE's hl pass must keep up with the
                # fp8-DR PE rate.
                tn = chunks[ch]
                p1 = ps1.tile([P, n_fo, tn], f32, name="p1", tag="p1")
                terms = [(0, 0), (0, KO), (KO, 2 * KO)]
                for j in range(n_fo):
                    b, half = (fo + j) // 2, (fo + j) % 2
                    n = 0
                    for wo, xo in terms:
                        for t in range(KO // 2):
                            nc.tensor.matmul(
                                p1[:, j, :],
                                w1sb[:, b, wo + 2 * t:wo + 2 * t + 2,
                                     half * P:(half + 1) * P],
                                xsbs[ch][:, xo + 2 * t:xo + 2 * t + 2, :],
                                start=(n == 0),
                                stop=(n == 3 * (KO // 2) - 1),
                                perf_mode=DR)
                            n += 1
                nc.scalar.activation(
                    hhs[ch][:, fo:fo + n_fo, :], p1[:],
                    mybir.ActivationFunctionType.Relu, bias=bias0[:])
                # hl = fp8(max(p1,0) - hh) on DVE (gpsimd's
                # scalar_tensor_tensor fails the walrus verifier)
                nc.vector.scalar_tensor_tensor(
                    hls[ch][:, fo:fo + n_fo, :], p1[:], 0.0,
                    hhs[ch][:, fo:fo + n_fo, :], AOP.max, AOP.subtract)

            def gemm2(ch, db, t0, tn, csl, q=None):
                # all three terms accumulate S-scaled in ONE PSUM tile
                # (host stores W2h*S and W2l*S — exact fp8 exponent
                # shifts); the single 1/S rescale rides the activation
                # engine's built-in output scale at eviction.
                p2 = ps2.tile([P, tn], f32, name="p2", tag="p2")
                for n, (hsb, wi) in enumerate(
                        ((hhs[ch], 0), (hls[ch], 0), (hhs[ch], 1))):
                    for g in range(FO // 2):
                        nc.tensor.matmul(
                            p2[:],
                            w2sb[:, db, wi, 2 * g:2 * g + 2, :],
                            hsb[:, 2 * g:2 * g + 2, t0:t0 + tn],
                            start=(g == 0 and n == 0),
                            stop=(g == FO // 2 - 1 and n == 2),
                            perf_mode=DR)
                ysb = ypool.tile([P, tn], bf16, tag="y", name="ysb")
                nc.scalar.mul(ysb[:], p2[:], 1.0 / S)
                # stores ride the SP queue (the scalar sequencer must
                # stay free for the evictions); the second-to-last store
                # is diverted to gpsimd/SWDGE so the last two don't
                # serialize on the HWDGE device
                (q or nc.sync).dma_start(yt.ap()[:, db, csl], ysb[:])

            for _ in range(repeat):
                # phase 1: quad-step-major with staggered chunk intro
                # (chunk ch joins at quad-step intro[ch] and catches up
                # on its backlog).
                # tiny chunks run in 8-fo groups (their PSUM tile is
                # still sub-bank) so they cost 2 evictions, not 16
                def emit_group(ch):
                    nf = 8 if chunks[ch] <= 64 else 2
                    gemm1_pair(ch, 2 * emitted[ch], nf)
                    emitted[ch] += nf // 2

                emitted = [0] * NCH
                for step in range(FO // 2):
                    for ch in range(NCH):
                        if intro[ch] <= step:
                            while emitted[ch] <= step:
                                emit_group(ch)
                for ch in range(NCH):
                    while emitted[ch] < FO // 2:
                        emit_group(ch)
                # phase 2: d-block-major, streamed out.  The final
                # d-block runs smallest chunks first so the last two
                # stores (the only ones with no compute left to hide
                # them) don't also queue on HWDGE behind a third.
                for db in range(KO):
                    order = list(range(NCH))
                    if db == KO - 1:
                        order.sort(key=lambda c: chunks[c])
                    for i, ch in enumerate(order):
                        tn = chunks[ch]
                        s0 = starts[ch]
                        # second-to-last store rides gpsimd/SWDGE so the
                        # last two stores don't serialize on HWDGE
                        q = (nc.gpsimd if db == KO - 1
                             and i == len(order) - 2 else None)
                        gemm2(ch, db, 0, tn, slice(s0, s0 + tn), q=q)

    nc.compile()
    _CACHE[key] = nc
    return nc


def _capacity(max_count):
    """C >= max_count, multiple of 8; 256-token chunks (a 2-fo PSUM
    tile fills one 2KB bank exactly) plus a small remainder chunk."""
    maxc = max(int(max_count), 16)
    c = -(-maxc // 8) * 8
    chunks = []
    while c > 256:
        chunks.append(256)
        c -= 256
    chunks.append(c)
    return tuple(chunks)


def _q8(x):
    return np.asarray(x, dtype=np.float32).astype(F8)


def _pack_expert(xT, w1, w2, chunks):
    """Quantize + lay out one expert's operands.

    xT: (d_model, C) f32 (scores folded).  Returns the in_map with the
    per-chunk xq* plus w1q / w2q tensors (see _build for dim layouts).
    """
    C = xT.shape[1]
    assert sum(chunks) == C
    xh_f = _q8(xT)
    xl_f = _q8(xT - xh_f.astype(np.float32))
    xs_f = _q8(xh_f.astype(np.float32) / S)

    def pack_x(m):  # (d_model, C) -> (P, KO, C)
        return m.reshape(KO, P, C).transpose(1, 0, 2)

    xq = np.concatenate(
        [pack_x(xh_f), pack_x(xl_f), pack_x(xs_f)], axis=1)
    in_map = {}
    s0 = 0
    for ch, tn in enumerate(chunks):
        in_map[f"xq{ch}"] = np.ascontiguousarray(xq[:, :, s0:s0 + tn])
        s0 += tn

    # w1_h[p, ko, f] = W1[f, ko*128+p]; blocked (P, NB, 2*KO, 256)
    w1T = w1.T.reshape(KO, P, D_FF).transpose(1, 0, 2)
    w1h_f = _q8(w1T)
    w1l_f = _q8(S * (w1T - w1h_f.astype(np.float32)))
    w1cat = np.concatenate([w1h_f, w1l_f], axis=1)  # (P, 8, D_FF)
    NB = D_FF // 256
    in_map["w1q"] = np.ascontiguousarray(
        w1cat.reshape(P, 2 * KO, NB, 256).transpose(0, 2, 1, 3))

    # w2_h[p, db, fo, q] = W2[db*128+q, fo*128+p]; both W2 operands are
    # stored pre-scaled by S (exact fp8 exponent shift) so GEMM2's
    # three terms share one PSUM accumulation; eviction divides by S.
    w2T = w2.T.reshape(FO, P, KO, P).transpose(1, 2, 0, 3)
    w2h_f = _q8(w2T)
    w2hs_f = _q8(S * w2h_f.astype(np.float32))
    w2l_f = _q8(S * (w2T - w2h_f.astype(np.float32)))
    in_map["w2q"] = np.ascontiguousarray(
        np.stack([w2hs_f, w2l_f], axis=2))  # (P, KO, 2, FO, P)

    return in_map


_last = {}


def kernel(inp, gate_idx, gate_score, w_htoh4, w_h4toh):
    inp = np.ascontiguousarray(np.asarray(inp, dtype=np.float32))
    gate_idx = np.asarray(gate_idx)
    gate_score = np.asarray(gate_score, dtype=np.float32)
    w_htoh4 = np.asarray(w_htoh4, dtype=np.float32)
    w_h4toh = np.asarray(w_h4toh, dtype=np.float32)

    B, d_model = inp.shape
    n_expert, d_ff, _ = w_htoh4.shape
    assert n_expert == NUM_EXPERT
    assert d_model == D_MODEL and d_ff == D_FF

    gi = gate_idx.astype(np.int64)
    order = np.argsort(gi, kind="stable")
    counts = np.bincount(gi, minlength=NUM_EXPERT)
    idx_split = np.split(order, np.cumsum(counts)[:-1])

    chunks = _capacity(counts.max())
    C = sum(chunks)

    scores_flat = gate_score.reshape(-1)

    nc = _build(chunks)

    in_maps = []
    for e in range(NUM_EXPERT):
        idx = idx_split[e]
        cnt = len(idx)
        xT = np.zeros((d_model, C), dtype=np.float32)
        if cnt:
            xT[:, :cnt] = (inp[idx] * scores_flat[idx][:, None]).T
        in_maps.append(_pack_expert(xT, w_htoh4[e], w_h4toh[e], chunks))

    from concourse import bass_utils
    res = bass_utils.run_bass_kernel_spmd(nc, in_maps,
                                          core_ids=list(range(N_CORES)))

    _last.update(nc=nc, in_maps=in_maps, res=res, chunks=chunks)

    y_full = np.empty((B, d_model), dtype=np.float32)
    for e in range(NUM_EXPERT):
        idx = idx_split[e]
        if len(idx) == 0:
            continue
        yt_h = np.asarray(res.results[e]["yt"], dtype=np.float32)  # (P,KO,C)
        yT = yt_h.transpose(1, 0, 2).reshape(d_model, C)
        y_full[idx] = yT[:, :len(idx)].T

    out = y_full[0::2] + y_full[1::2]
    return np.ascontiguousarray(out, dtype=np.float32)
